# revision 28
# baseline (speedup 1.0000x reference)
"""Trainium2 Bass kernel for a 3-layer GraphSAGE GNN (mean aggregation + BN + ReLU).

Self-contained: kernel(**inputs) -> np.ndarray [50000, 128] float32.

Strategy (8 NeuronCores, SPMD):
  - Nodes sharded 8 ways (6272/core = 49 tiles of 128). Edges assigned to the
    core owning their destination, sorted by dst, binned per 128-node tile.
  - Per-edge source features fetched with dma_gather from a per-core HBM table
    (bf16, 256B rows). Each SWDGE queue is an independent ~33GB/s pipe
    (latency-bound), so each chunk's gathers are split across the 4 queues
    (~130GB/s aggregate). Chunks are 4 tiles (1-tile chunk first).
  - Segment-mean via one-hot matmuls on TensorE; the one-hot S matrices
    (1/deg folded in) are precomputed on host and streamed from HBM.
  - BN+ReLU is deferred across the AllGather: the table holds PRE-BN y;
    since s = g/sigma > 0, relu(s*y + t) = s*relu(y + t/s), so the kernel
    stores y, applies max(y + theta, 0) per gathered msg chunk on the Vector
    engine (theta = t/s, replicated via TensorE outer product), and folds s
    into the next layer's weight matrices on device. This removes the serial
    AllReduce -> apply -> AllGather chain from the layer boundary: AllGather
    is issued immediately at layer end, stats AllReduce right after it.
  - Shard transposes + writes happen per-chunk during the layer; the dense
    part applies the bias in the ACT stage which also emits BN statistics.
"""
import sys, types
import numpy as np
import ml_dtypes

BF16 = ml_dtypes.bfloat16

# ---------------- problem constants (hardcoded per the task) ----------------
N = 50000
E = 800000
FIN = 3
H = 64
OUT = 128
NCORES = 8
P = 128                 # partitions / node tile
TILES = 49              # tiles per core
SH = TILES * P          # 6272 nodes per core (padded)
NTAB = NCORES * SH      # 50176 table rows
SPLIT = NTAB // 2       # 25088: lo rows [0, SPLIT), hi rows [SPLIT, NTAB)
EPS = 1e-5
NQ = 4                  # SWDGE queues for gathers
SCRATCH = 24576         # dynamic DMA scratch (descriptor rings), bytes/partition
# 1-tile chunk first (tiny gathers issue into empty queues; avoids a late
# straggler at the layer tail), then twelve 4-tile chunks.
CHUNKS = [(48, 49)] + [(4 * i, 4 * i + 4) for i in range(12)]

_CACHE = {}
SKIP_COLLECTIVES = False


def _install_ntff_shim():
    import antenv
    if hasattr(antenv, "axon_hooks"):
        return
    mod = types.ModuleType("antenv.axon_hooks")
    _hook = [None]
    mod.set_axon_ntff_profile_hook = lambda h: _hook.__setitem__(0, h)
    mod.get_axon_ntff_profile_hook = lambda: _hook[0]
    sys.modules["antenv.axon_hooks"] = mod
    antenv.axon_hooks = mod
    try:
        from trn_agent_boot.trn_boot import _ntff_profile_via_ctypes
        h = _ntff_profile_via_ctypes("/opt/axon/libaxon_pjrt.so")
        if h is not None:
            mod.set_axon_ntff_profile_hook(h)
    except Exception:
        pass


# ---------------------------- host preprocessing ----------------------------

def _wrap_idx(arr):
    """int16 position-i -> partition i%16, col i//16; replicated to 128 parts."""
    n = arr.shape[0]
    assert n % 16 == 0
    base = arr.reshape(n // 16, 16).T.astype(np.int16)      # [16, n/16]
    return np.tile(base, (8, 1))                            # [128, n/16]


def _prep(edge_index):
    """Per-core gather/selection structures. Returns (B2L, B2H, per_core list)."""
    src = edge_index[0].astype(np.int64)
    dst = edge_index[1].astype(np.int64)

    deg = np.bincount(dst, minlength=N).astype(np.float64)
    invdeg = (1.0 / np.maximum(deg, 1.0)).astype(np.float32)        # [N]

    core = dst // SH                                   # owning core per edge
    tile = (dst % SH) // P                             # tile within core
    loc = dst % P                                      # dst offset within tile
    ishi = (src >= SPLIT).astype(np.int64)

    # group key: (core, tile, ishi); count per group to fix block counts
    key = (core * TILES + tile) * 2 + ishi
    counts = np.bincount(key, minlength=NCORES * TILES * 2)
    cnt_lo = counts[0::2].reshape(NCORES, TILES)
    cnt_hi = counts[1::2].reshape(NCORES, TILES)
    B2L = int(np.ceil(cnt_lo.max() / P))
    B2H = int(np.ceil(cnt_hi.max() / P))
    NB = B2L + B2H

    # stable ordering of edges by group
    order = np.argsort(key, kind="stable")
    ksort = key[order]
    starts = np.searchsorted(ksort, np.arange(NCORES * TILES * 2))
    ends = np.append(starts[1:], len(order))

    per_core = []
    for k in range(NCORES):
        nlo = TILES * B2L * P
        nhi = TILES * B2H * P
        idx_lo = np.zeros(nlo, np.int64)
        idx_hi = np.zeros(nhi, np.int64)
        sel_lo = np.full(nlo, -1, np.int64)
        sel_hi = np.full(nhi, -1, np.int64)
        for t in range(TILES):
            g = (k * TILES + t) * 2
            for hi in (0, 1):
                idxs = order[starts[g + hi]:ends[g + hi]]
                c = len(idxs)
                if hi:
                    base = t * B2H * P
                    idx_hi[base:base + c] = src[idxs] - SPLIT
                    sel_hi[base:base + c] = loc[idxs]
                else:
                    base = t * B2L * P
                    idx_lo[base:base + c] = src[idxs]
                    sel_lo[base:base + c] = loc[idxs]
        # slot i -> output partition i%128, block i//128 (dma_gather layout)
        slo = sel_lo.reshape(TILES * B2L, P).T              # [P, TILES*B2L]
        shi = sel_hi.reshape(TILES * B2H, P).T              # [P, TILES*B2H]
        # dsel in chunk order: tile t, block b (lo blocks then hi blocks)
        dselc = np.empty((P, TILES * NB), np.int64)
        for t in range(TILES):
            dselc[:, t * NB:t * NB + B2L] = slo[:, t * B2L:(t + 1) * B2L]
            dselc[:, t * NB + B2L:(t + 1) * NB] = shi[:, t * B2H:(t + 1) * B2H]
        # host-built one-hot S with 1/deg folded in:
        #   S[p, j, c] = invdeg[node(tile(j), c)] if dselc[p, j] == c else 0
        lo_n = k * SH
        hi_n = min((k + 1) * SH, N)
        iv = np.ones(SH, np.float32)
        iv[: hi_n - lo_n] = invdeg[lo_n:hi_n]
        S = np.zeros((P, TILES * NB, P), BF16)
        pidx, jidx = np.nonzero(dselc >= 0)
        cidx = dselc[pidx, jidx]
        tl = jidx // NB
        S[pidx, jidx, cidx] = iv[tl * P + cidx].astype(BF16)
        d = dict(
            idx_lo=_wrap_idx(idx_lo.astype(np.int16)),
            idx_hi=_wrap_idx(idx_hi.astype(np.int16)),
            S=np.ascontiguousarray(S.reshape(P, TILES * NB * P)),
        )
        d["shard_len"] = hi_n - lo_n
        per_core.append(d)
    return B2L, B2H, per_core


# ------------------------------- bass program -------------------------------

def _build(B2L, B2H):
    import concourse.bass as bass
    import concourse.bacc as bacc
    import concourse.tile as tile
    import concourse.mybir as mybir

    dt = mybir.dt
    Alu = mybir.AluOpType
    Act = mybir.ActivationFunctionType

    nc = bacc.Bacc("TRN2", target_bir_lowering=False, debug=False,
                   num_devices=NCORES, num_swdge_queues=NQ,
                   dynamic_dma_scratch_size=SCRATCH)

    NB = B2L + B2H

    # ---------------- I/O ----------------
    def inp(name, shape, d):
        return nc.dram_tensor(name, list(shape), d, kind="ExternalInput")

    table0 = inp("table0", [NTAB, P], dt.bfloat16)
    xTown = inp("xTown", [FIN, SH], dt.bfloat16)
    idx_lo = inp("idx_lo", [P, TILES * B2L * 8], dt.int16)
    idx_hi = inp("idx_hi", [P, TILES * B2H * 8], dt.int16)
    S_dram = inp("S", [P, TILES * NB * P], dt.bfloat16)
    identb = inp("identb", [H, H], dt.bfloat16)
    ones1 = inp("ones1", [1, P], dt.bfloat16)
    Wl0b = inp("Wl0b", [FIN, H], dt.bfloat16)
    Wl1b = inp("Wl1b", [H, H], dt.bfloat16)
    Wl2b = inp("Wl2b", [H, OUT], dt.bfloat16)
    Wr0b = inp("Wr0b", [FIN, H], dt.bfloat16)
    Wr1b = inp("Wr1b", [H, H], dt.bfloat16)
    Wr2b = inp("Wr2b", [H, OUT], dt.bfloat16)
    bl0c = inp("bl0c", [H, 1], dt.float32)
    bl1c = inp("bl1c", [H, 1], dt.float32)
    bl2c = inp("bl2c", [OUT, 1], dt.float32)
    g0c = inp("g0c", [H, 1], dt.float32)
    b0c = inp("b0c", [H, 1], dt.float32)
    g1c = inp("g1c", [H, 1], dt.float32)
    b1c = inp("b1c", [H, 1], dt.float32)

    out = nc.dram_tensor("out", [OUT, SH], dt.float32, kind="ExternalOutput")

    # DRAM scratch
    shard0 = nc.dram_tensor("shard0", [SH, P], dt.bfloat16)
    shard1 = nc.dram_tensor("shard1", [SH, P], dt.bfloat16)
    table1 = nc.dram_tensor("table1", [NTAB, P], dt.bfloat16, addr_space="Shared")
    table2 = nc.dram_tensor("table2", [NTAB, P], dt.bfloat16, addr_space="Shared")
    stats_in0 = nc.dram_tensor("stats_in0", [H, 2], dt.float32)
    stats_in1 = nc.dram_tensor("stats_in1", [H, 2], dt.float32)
    stats_out0 = nc.dram_tensor("stats_out0", [H, 2], dt.float32, addr_space="Shared")
    stats_out1 = nc.dram_tensor("stats_out1", [H, 2], dt.float32, addr_space="Shared")
    warm_in = nc.dram_tensor("warm_in", [H, 2], dt.float32)
    warm_out = nc.dram_tensor("warm_out", [H, 2], dt.float32, addr_space="Shared")

    layers = [
        dict(table=table0, Wl=Wl0b, Wr=Wr0b, bl=bl0c, KA=FIN,
             g=g0c, b=b0c, HO=H, sin=stats_in0, sout=stats_out0,
             shard=shard0, tnext=table1),
        dict(table=table1, Wl=Wl1b, Wr=Wr1b, bl=bl1c, KA=H,
             g=g1c, b=b1c, HO=H, sin=stats_in1, sout=stats_out1,
             shard=shard1, tnext=table2),
        dict(table=table2, Wl=Wl2b, Wr=Wr2b, bl=bl2c, KA=H,
             g=None, b=None, HO=OUT, sin=None, sout=None,
             shard=None, tnext=None),
    ]

    def allreduce(src, dst):
        nc.gpsimd.collective_compute(
            "AllReduce", Alu.add, replica_groups=[list(range(NCORES))],
            ins=[src.ap().opt()], outs=[dst.ap().opt()])

    with tile.TileContext(nc) as tc:
        with tc.tile_pool(name="const", bufs=1) as cpool, \
             tc.tile_pool(name="work", bufs=3) as wpool, \
             tc.tile_pool(name="outp", bufs=2) as opool, \
             tc.tile_pool(name="msgp", bufs=3) as mpool, \
             tc.tile_pool(name="msgt", bufs=1) as mtpool, \
             tc.tile_pool(name="spool", bufs=3) as spool, \
             tc.tile_pool(name="spoolt", bufs=1) as spoolt, \
             tc.tile_pool(name="psum", bufs=2, space="PSUM") as pp, \
             tc.tile_pool(name="psumb", bufs=1, space="PSUM") as ppb:

            def load_const(t):
                sl = tuple(slice(0, s) for s in t.shape)
                tl = cpool.tile(list(t.shape), t.dtype, tag=t.name,
                                name=f"c_{t.name}")
                nc.sync.dma_start(out=tl[:], in_=t[sl])
                return tl

            xTown_t = load_const(xTown)
            idxlo_t = load_const(idx_lo)
            idxhi_t = load_const(idx_hi)
            identb_t = load_const(identb)
            ones1_t = load_const(ones1)
            W_t = {l: tuple(load_const(t) for t in ts)
                   for l, ts in {0: (Wl0b, Wr0b, bl0c),
                                 1: (Wl1b, Wr1b, bl1c),
                                 2: (Wl2b, Wr2b, bl2c)}.items()}
            bn_t = {0: (load_const(g0c), load_const(b0c)),
                    1: (load_const(g1c), load_const(b1c))}

            # collective warmup (result unused; overlaps layer-0 gathers)
            if not SKIP_COLLECTIVES:
                allreduce(warm_in, warm_out)

            # persistent regions
            yT_l = {0: cpool.tile([H, SH], dt.bfloat16, tag="yTA",
                                  name="yTA"),
                    1: cpool.tile([H, SH], dt.bfloat16, tag="yTB",
                                  name="yTB")}
            NCH = len(CHUNKS)
            ssum = cpool.tile([H, NCH], dt.float32, tag="ssum")
            ssq = cpool.tile([H, NCH], dt.float32, tag="ssq")
            # deferred-BN state produced at each boundary (l = 0, 1)
            bn_state = {}

            for l, L in enumerate(layers):
                table = L["table"]
                Wl_t, Wr_t, bl_t = W_t[l]
                KA = L["KA"]
                HO = L["HO"]
                qctr = 0
                if l > 0:
                    st = bn_state[l - 1]
                for (t0, t1) in CHUNKS:
                    ct = t1 - t0
                    pool = mtpool if ct == 1 else mpool
                    msgL = pool.tile([P, ct * B2L, P], dt.bfloat16,
                                     tag=f"msgL{ct}")
                    msgH = pool.tile([P, ct * B2H, P], dt.bfloat16,
                                     tag=f"msgH{ct}")
                    # gathers split in half per lo/hi, spread over 4 queues
                    for (msg, idxs_t, bpt) in ((msgL, idxlo_t, B2L),
                                               (msgH, idxhi_t, B2H)):
                        nblk_tot = ct * bpt
                        hsp = [(0, nblk_tot)] if ct == 1 else \
                            [(0, nblk_tot // 2), (nblk_tot // 2, nblk_tot)]
                        for (b0_, b1_) in hsp:
                            nb = b1_ - b0_
                            ni = nb * P
                            i0 = (t0 * bpt + b0_) * 8
                            nc.gpsimd.dma_gather(
                                out_ap=msg[:, b0_:b1_, :],
                                in_ap=(table[0:SPLIT, :] if msg is msgL
                                       else table[SPLIT:NTAB, :]),
                                idxs_ap=idxs_t[:, i0:i0 + nb * 8],
                                num_idxs=ni, num_idxs_reg=ni, elem_size=P,
                                single_packet=False, queue_num=qctr % NQ)
                            qctr += 1
                            if l > 0:
                                # deferred BN+relu per gathered half:
                                #   msg = max(msg + theta, 0) on cols 0:KA
                                st_ = bn_state[l - 1]
                                th_b = st_["th_rep"][:, 0:KA].unsqueeze(1) \
                                    .broadcast_to([P, nb, KA])
                                nc.vector.tensor_tensor(
                                    out=msg[:, b0_:b1_, 0:KA],
                                    in0=msg[:, b0_:b1_, 0:KA],
                                    in1=th_b, op=Alu.add)
                                nc.vector.tensor_scalar(
                                    out=msg[:, b0_:b1_, 0:KA],
                                    in0=msg[:, b0_:b1_, 0:KA],
                                    scalar1=0.0, scalar2=None, op0=Alu.max)
                    # one-hot S (invdeg folded), streamed from HBM
                    St = (spool if ct > 1 else spoolt).tile(
                        [P, ct * NB, P], dt.bfloat16, tag=f"S{ct}",
                        name=f"S{ct}")
                    nc.sync.dma_start(
                        out=St[:],
                        in_=S_dram[:, t0 * NB * P:t1 * NB * P])
                    if l > 0:
                        # own-side: relu(y_prev + theta) feature-major
                        own_c = wpool.tile([H, ct * P], dt.bfloat16,
                                           tag=f"own{ct}")
                        nc.scalar.activation(
                            out=own_c[:], in_=yT_l[l - 1][:, t0 * P:t1 * P],
                            func=Act.Relu, bias=st["th_col"][:, 0:1])
                        Wl_use, Wr_use = st["Wlp"], st["Wrp"]
                    else:
                        Wl_use, Wr_use = Wl_t, Wr_t
                    ci = CHUNKS.index((t0, t1))
                    # batched PSUM: all ct tiles' agg chains in one bank
                    aggc = pp.tile([P, 4 * P], dt.float32, tag="aggc")
                    for t in range(t0, t1):
                        tt = t - t0          # chunk-local tile index
                        for b in range(NB):
                            if b < B2L:
                                mblk = msgL[:, tt * B2L + b, 0:KA]
                            else:
                                mblk = msgH[:, tt * B2H + (b - B2L), 0:KA]
                            nc.tensor.matmul(
                                out=aggc[0:KA, tt * P:(tt + 1) * P],
                                lhsT=mblk, rhs=St[:, tt * NB + b, :],
                                start=(b == 0), stop=(b == NB - 1))
                    # cast (mean already folded into S), one op per chunk
                    aggb = wpool.tile([KA, ct * P], dt.bfloat16,
                                      tag=f"aggb{KA}_{ct}",
                                      name=f"aggb{KA}_{ct}")
                    nc.vector.tensor_copy(out=aggb[:],
                                          in_=aggc[0:KA, 0:ct * P])
                    # dense: yT = Wl^T aggb + Wr^T own (+ bias in ACT)
                    ypsc = pp.tile([OUT, 4 * P], dt.float32, tag="ypsc")
                    for t in range(t0, t1):
                        tt = t - t0
                        nc.tensor.matmul(out=ypsc[0:HO, tt * P:(tt + 1) * P],
                                         lhsT=Wl_use[:],
                                         rhs=aggb[:, tt * P:(tt + 1) * P],
                                         start=True, stop=False)
                        rhs_own = (own_c[:, tt * P:(tt + 1) * P] if l > 0
                                   else xTown_t[:, t * P:(t + 1) * P])
                        nc.tensor.matmul(out=ypsc[0:HO, tt * P:(tt + 1) * P],
                                         lhsT=Wr_use[:], rhs=rhs_own,
                                         start=False, stop=True)
                    if l < 2:
                        yT = yT_l[l]
                        nc.scalar.activation(
                            out=yT[:, t0 * P:t1 * P],
                            in_=ypsc[0:H, 0:ct * P], func=Act.Identity,
                            bias=bl_t[:, 0:1],
                            accum_out=ssum[:, ci:ci + 1])
                        sq = wpool.tile([H, ct * P], dt.float32,
                                        tag=f"sq{ct}")
                        nc.scalar.activation(
                            out=sq[:], in_=ypsc[0:H, 0:ct * P], func=Act.Square,
                            bias=bl_t[:, 0:1],
                            accum_out=ssq[:, ci:ci + 1])
                        # transpose + shard write (pre-BN y), per chunk
                        y_chunk = opool.tile([P, ct, P], dt.bfloat16,
                                             tag=f"yc{ct}")
                        nc.vector.memset(y_chunk[:, :, H:P], 0.0)
                        for t in range(t0, t1):
                            ptr = pp.tile([P, H], dt.bfloat16, tag="ps_tr")
                            nc.tensor.transpose(
                                out=ptr[:], in_=yT[:, t * P:(t + 1) * P],
                                identity=identb_t[:])
                            nc.scalar.activation(
                                out=y_chunk[:, t - t0, 0:H], in_=ptr[:],
                                func=Act.Copy)
                        nc.scalar.dma_start(
                            out=L["shard"][t0 * P:t1 * P, :].rearrange(
                                "(t p) d -> p t d", p=P),
                            in_=y_chunk[:])
                    else:
                        y2c = opool.tile([OUT, ct * P], dt.float32,
                                         tag=f"y2c{ct}")
                        nc.scalar.activation(
                            out=y2c[:], in_=ypsc[0:OUT, 0:ct * P],
                            func=Act.Identity, bias=bl_t[:, 0:1])
                        nc.scalar.dma_start(out=out[:, t0 * P:t1 * P],
                                            in_=y2c[:])

                if l < 2:
                    # ---- table AllGather first (no stats dependency), then
                    # ---- BN stats AllReduce right behind it on the CC stream
                    if SKIP_COLLECTIVES:
                        nc.sync.dma_start(out=L["tnext"][0:SH, 0:P],
                                          in_=L["shard"][0:SH, 0:P])
                    else:
                        nc.gpsimd.collective_compute(
                            "AllGather", Alu.bypass,
                            replica_groups=[list(range(NCORES))],
                            ins=[L["shard"].ap().opt()],
                            outs=[L["tnext"].ap().opt()])
                    stats = wpool.tile([H, 2], dt.float32, tag="stats")
                    nc.vector.tensor_reduce(out=stats[:, 0:1], in_=ssum[:],
                                            axis=mybir.AxisListType.X, op=Alu.add)
                    nc.vector.tensor_reduce(out=stats[:, 1:2], in_=ssq[:],
                                            axis=mybir.AxisListType.X, op=Alu.add)
                    nc.sync.dma_start(out=L["sin"][0:H, 0:2], in_=stats[:])
                    if SKIP_COLLECTIVES:
                        nc.sync.dma_start(out=L["sout"][0:H, 0:2],
                                          in_=L["sin"][0:H, 0:2])
                    else:
                        allreduce(L["sin"], L["sout"])
                    sg = wpool.tile([H, 2], dt.float32, tag="sg")
                    nc.sync.dma_start(out=sg[:], in_=L["sout"][0:H, 0:2])
                    # s = g/sqrt(var+eps); t = b - mu*s; theta = t/s
                    mu = wpool.tile([H, 1], dt.float32, tag="mu")
                    nc.vector.tensor_scalar(out=mu[:], in0=sg[:, 0:1],
                                            scalar1=1.0 / N, scalar2=None,
                                            op0=Alu.mult)
                    var = wpool.tile([H, 1], dt.float32, tag="var")
                    nc.vector.tensor_scalar(out=var[:], in0=sg[:, 1:2],
                                            scalar1=1.0 / N, scalar2=None,
                                            op0=Alu.mult)
                    mu2 = wpool.tile([H, 1], dt.float32, tag="mu2")
                    nc.vector.tensor_tensor(out=mu2[:], in0=mu[:], in1=mu[:],
                                            op=Alu.mult)
                    nc.vector.tensor_tensor(out=var[:], in0=var[:], in1=mu2[:],
                                            op=Alu.subtract)
                    nc.vector.tensor_scalar(out=var[:], in0=var[:],
                                            scalar1=float(EPS), scalar2=None,
                                            op0=Alu.add)
                    std = wpool.tile([H, 1], dt.float32, tag="std")
                    nc.scalar.activation(out=std[:], in_=var[:], func=Act.Sqrt)
                    istd = wpool.tile([H, 1], dt.float32, tag="istd")
                    nc.vector.reciprocal(out=istd[:], in_=std[:])
                    g_t, bb_t = bn_t[l]
                    s_col = cpool.tile([H, 1], dt.float32, tag=f"s_col{l}")
                    nc.vector.tensor_tensor(out=s_col[:], in0=g_t[:],
                                            in1=istd[:], op=Alu.mult)
                    ms = wpool.tile([H, 1], dt.float32, tag="ms")
                    nc.vector.tensor_tensor(out=ms[:], in0=mu[:], in1=s_col[:],
                                            op=Alu.mult)
                    t_col = wpool.tile([H, 1], dt.float32, tag="t_col")
                    nc.vector.tensor_tensor(out=t_col[:], in0=bb_t[:],
                                            in1=ms[:], op=Alu.subtract)
                    sinv = wpool.tile([H, 1], dt.float32, tag="sinv")
                    nc.vector.reciprocal(out=sinv[:], in_=s_col[:])
                    th_col = cpool.tile([H, 1], dt.float32, tag=f"th_col{l}")
                    nc.vector.tensor_tensor(out=th_col[:], in0=t_col[:],
                                            in1=sinv[:], op=Alu.mult)
                    # theta replicated to all partitions: ones1^T (theta^T)
                    th_cb = wpool.tile([H, 1], dt.bfloat16, tag="th_cb")
                    nc.vector.tensor_copy(out=th_cb[:], in_=th_col[:])
                    trps = ppb.tile([1, H], dt.float32, tag="trps")
                    nc.tensor.matmul(out=trps[:], lhsT=th_cb[:],
                                     rhs=identb_t[:], start=True, stop=True)
                    th_row = wpool.tile([1, H], dt.bfloat16, tag="th_row")
                    nc.vector.tensor_copy(out=th_row[:], in_=trps[:])
                    threp_ps = ppb.tile([P, H], dt.float32, tag="threp_ps")
                    nc.tensor.matmul(out=threp_ps[:], lhsT=ones1_t[:],
                                     rhs=th_row[:], start=True, stop=True)
                    th_rep = cpool.tile([P, H], dt.bfloat16, tag=f"th_rep{l}")
                    nc.vector.tensor_copy(out=th_rep[:], in_=threp_ps[:])
                    # fold s into the next layer's weights (rows = features)
                    Wln, Wrn, _ = W_t[l + 1]
                    HO_n = layers[l + 1]["HO"]
                    Wlp = cpool.tile([H, HO_n], dt.bfloat16, tag=f"Wlp{l}")
                    nc.vector.tensor_scalar(out=Wlp[:], in0=Wln[:],
                                            scalar1=s_col[:, 0:1],
                                            scalar2=None, op0=Alu.mult)
                    Wrp = cpool.tile([H, HO_n], dt.bfloat16, tag=f"Wrp{l}")
                    nc.vector.tensor_scalar(out=Wrp[:], in0=Wrn[:],
                                            scalar1=s_col[:, 0:1],
                                            scalar2=None, op0=Alu.mult)
                    bn_state[l] = dict(th_col=th_col, th_rep=th_rep,
                                       Wlp=Wlp, Wrp=Wrp)

    nc.compile()
    return nc


# --------------------------------- runner -----------------------------------

def _get_nc(B2L, B2H):
    key = (B2L, B2H)
    if key not in _CACHE:
        _CACHE[key] = _build(B2L, B2H)
    return _CACHE[key]


def make_in_maps(x, Wl0, bl0, Wr0, g0, b0, Wl1, bl1, Wr1, g1, b1,
                 Wl2, bl2, Wr2, per_core):
    x = np.asarray(x, np.float32)
    tab0 = np.zeros((NTAB, P), np.float32)
    tab0[:N, :FIN] = x
    tab0 = tab0.astype(BF16)
    xTfull = np.zeros((FIN, NTAB), np.float32)
    xTfull[:, :N] = x.T
    xTb = xTfull.astype(BF16)

    common = dict(
        table0=tab0,
        identb=np.eye(H, dtype=np.float32).astype(BF16),
        ones1=np.ones((1, P), np.float32).astype(BF16),
        Wl0b=np.asarray(Wl0, np.float32).astype(BF16),
        Wl1b=np.asarray(Wl1, np.float32).astype(BF16),
        Wl2b=np.asarray(Wl2, np.float32).astype(BF16),
        Wr0b=np.asarray(Wr0, np.float32).astype(BF16),
        Wr1b=np.asarray(Wr1, np.float32).astype(BF16),
        Wr2b=np.asarray(Wr2, np.float32).astype(BF16),
        bl0c=np.ascontiguousarray(np.asarray(bl0, np.float32).reshape(H, 1)),
        bl1c=np.ascontiguousarray(np.asarray(bl1, np.float32).reshape(H, 1)),
        bl2c=np.ascontiguousarray(np.asarray(bl2, np.float32).reshape(OUT, 1)),
        g0c=np.ascontiguousarray(np.asarray(g0, np.float32).reshape(H, 1)),
        b0c=np.ascontiguousarray(np.asarray(b0, np.float32).reshape(H, 1)),
        g1c=np.ascontiguousarray(np.asarray(g1, np.float32).reshape(H, 1)),
        b1c=np.ascontiguousarray(np.asarray(b1, np.float32).reshape(H, 1)),
        warm_in=np.zeros((H, 2), np.float32),
    )

    in_maps = []
    for k in range(NCORES):
        d = per_core[k]
        m = dict(common)
        m["xTown"] = np.ascontiguousarray(xTb[:, k * SH:(k + 1) * SH])
        for key in ("idx_lo", "idx_hi", "S"):
            m[key] = d[key]
        in_maps.append(m)
    return in_maps


def run(inputs, trace=False):
    """Build+run; returns (full_output, BassKernelResults)."""
    _install_ntff_shim()
    from concourse import bass_utils

    edge_index = np.asarray(inputs["edge_index"])
    B2L, B2H, per_core = _prep(edge_index)
    nc = _get_nc(B2L, B2H)
    in_maps = make_in_maps(
        inputs["x"], inputs["Wl0"], inputs["bl0"], inputs["Wr0"],
        inputs["g0"], inputs["b0"], inputs["Wl1"], inputs["bl1"],
        inputs["Wr1"], inputs["g1"], inputs["b1"], inputs["Wl2"],
        inputs["bl2"], inputs["Wr2"], per_core)
    res = bass_utils.run_bass_kernel_spmd(nc, in_maps,
                                          core_ids=list(range(NCORES)),
                                          trace=trace)
    parts = []
    for k in range(NCORES):
        n_k = per_core[k]["shard_len"]
        parts.append(res.results[k]["out"][:, :n_k].T)
    full = np.ascontiguousarray(np.concatenate(parts, axis=0),
                                dtype=np.float32)
    return full, res


def kernel(x, edge_index, Wl0, bl0, Wr0, g0, b0, Wl1, bl1, Wr1, g1, b1,
           Wl2, bl2, Wr2):
    full, _ = run(dict(x=x, edge_index=edge_index, Wl0=Wl0, bl0=bl0, Wr0=Wr0,
                       g0=g0, b0=b0, Wl1=Wl1, bl1=bl1, Wr1=Wr1, g1=g1, b1=b1,
                       Wl2=Wl2, bl2=bl2, Wr2=Wr2))
    return full


# revision 29
# speedup vs baseline: 1.0023x; 1.0023x over previous
"""Trainium2 Bass kernel for a 3-layer GraphSAGE GNN (mean aggregation + BN + ReLU).

Self-contained: kernel(**inputs) -> np.ndarray [50000, 128] float32.

Strategy (8 NeuronCores, SPMD):
  - Nodes sharded 8 ways (6272/core = 49 tiles of 128). Edges assigned to the
    core owning their destination, sorted by dst, binned per 128-node tile.
  - Per-edge source features fetched with dma_gather from a per-core HBM table
    (bf16, 256B rows). Each SWDGE queue is an independent ~33GB/s pipe
    (latency-bound), so each chunk's gathers are split across the 4 queues
    (~130GB/s aggregate). Chunks are 4 tiles (1-tile chunk first).
  - Segment-mean via one-hot matmuls on TensorE; the one-hot S matrices
    (1/deg folded in) are precomputed on host and streamed from HBM.
  - BN+ReLU is deferred across the AllGather: the table holds PRE-BN y;
    since s = g/sigma > 0, relu(s*y + t) = s*relu(y + t/s), so the kernel
    stores y, applies max(y + theta, 0) per gathered msg chunk on the Vector
    engine (theta = t/s, replicated via TensorE outer product), and folds s
    into the next layer's weight matrices on device. This removes the serial
    AllReduce -> apply -> AllGather chain from the layer boundary: AllGather
    is issued immediately at layer end, stats AllReduce right after it.
  - Shard transposes + writes happen per-chunk during the layer; the dense
    part applies the bias in the ACT stage which also emits BN statistics.
"""
import sys, types
import numpy as np
import ml_dtypes

BF16 = ml_dtypes.bfloat16

# ---------------- problem constants (hardcoded per the task) ----------------
N = 50000
E = 800000
FIN = 3
H = 64
OUT = 128
NCORES = 8
P = 128                 # partitions / node tile
TILES = 49              # tiles per core
SH = TILES * P          # 6272 nodes per core (padded)
NTAB = NCORES * SH      # 50176 table rows
SPLIT = NTAB // 2       # 25088: lo rows [0, SPLIT), hi rows [SPLIT, NTAB)
EPS = 1e-5
NQ = 4                  # SWDGE queues for gathers
SCRATCH = 32768         # dynamic DMA scratch (descriptor rings), bytes/partition
# 1-tile chunk first (tiny gathers issue into empty queues; avoids a late
# straggler at the layer tail), then twelve 4-tile chunks.
CHUNKS = [(48, 49)] + [(4 * i, 4 * i + 4) for i in range(12)]

_CACHE = {}
SKIP_COLLECTIVES = False


def _install_ntff_shim():
    import antenv
    if hasattr(antenv, "axon_hooks"):
        return
    mod = types.ModuleType("antenv.axon_hooks")
    _hook = [None]
    mod.set_axon_ntff_profile_hook = lambda h: _hook.__setitem__(0, h)
    mod.get_axon_ntff_profile_hook = lambda: _hook[0]
    sys.modules["antenv.axon_hooks"] = mod
    antenv.axon_hooks = mod
    try:
        from trn_agent_boot.trn_boot import _ntff_profile_via_ctypes
        h = _ntff_profile_via_ctypes("/opt/axon/libaxon_pjrt.so")
        if h is not None:
            mod.set_axon_ntff_profile_hook(h)
    except Exception:
        pass


# ---------------------------- host preprocessing ----------------------------

def _wrap_idx(arr):
    """int16 position-i -> partition i%16, col i//16; replicated to 128 parts."""
    n = arr.shape[0]
    assert n % 16 == 0
    base = arr.reshape(n // 16, 16).T.astype(np.int16)      # [16, n/16]
    return np.tile(base, (8, 1))                            # [128, n/16]


def _prep(edge_index):
    """Per-core gather/selection structures. Returns (B2L, B2H, per_core list)."""
    src = edge_index[0].astype(np.int64)
    dst = edge_index[1].astype(np.int64)

    deg = np.bincount(dst, minlength=N).astype(np.float64)
    invdeg = (1.0 / np.maximum(deg, 1.0)).astype(np.float32)        # [N]

    core = dst // SH                                   # owning core per edge
    tile = (dst % SH) // P                             # tile within core
    loc = dst % P                                      # dst offset within tile
    ishi = (src >= SPLIT).astype(np.int64)

    # group key: (core, tile, ishi); count per group to fix block counts
    key = (core * TILES + tile) * 2 + ishi
    counts = np.bincount(key, minlength=NCORES * TILES * 2)
    cnt_lo = counts[0::2].reshape(NCORES, TILES)
    cnt_hi = counts[1::2].reshape(NCORES, TILES)
    B2L = int(np.ceil(cnt_lo.max() / P))
    B2H = int(np.ceil(cnt_hi.max() / P))
    NB = B2L + B2H

    # stable ordering of edges by group
    order = np.argsort(key, kind="stable")
    ksort = key[order]
    starts = np.searchsorted(ksort, np.arange(NCORES * TILES * 2))
    ends = np.append(starts[1:], len(order))

    per_core = []
    for k in range(NCORES):
        nlo = TILES * B2L * P
        nhi = TILES * B2H * P
        idx_lo = np.zeros(nlo, np.int64)
        idx_hi = np.zeros(nhi, np.int64)
        sel_lo = np.full(nlo, -1, np.int64)
        sel_hi = np.full(nhi, -1, np.int64)
        for t in range(TILES):
            g = (k * TILES + t) * 2
            for hi in (0, 1):
                idxs = order[starts[g + hi]:ends[g + hi]]
                c = len(idxs)
                if hi:
                    base = t * B2H * P
                    idx_hi[base:base + c] = src[idxs] - SPLIT
                    sel_hi[base:base + c] = loc[idxs]
                else:
                    base = t * B2L * P
                    idx_lo[base:base + c] = src[idxs]
                    sel_lo[base:base + c] = loc[idxs]
        # slot i -> output partition i%128, block i//128 (dma_gather layout)
        slo = sel_lo.reshape(TILES * B2L, P).T              # [P, TILES*B2L]
        shi = sel_hi.reshape(TILES * B2H, P).T              # [P, TILES*B2H]
        # dsel in chunk order: tile t, block b (lo blocks then hi blocks)
        dselc = np.empty((P, TILES * NB), np.int64)
        for t in range(TILES):
            dselc[:, t * NB:t * NB + B2L] = slo[:, t * B2L:(t + 1) * B2L]
            dselc[:, t * NB + B2L:(t + 1) * NB] = shi[:, t * B2H:(t + 1) * B2H]
        # host-built one-hot S with 1/deg folded in:
        #   S[p, j, c] = invdeg[node(tile(j), c)] if dselc[p, j] == c else 0
        lo_n = k * SH
        hi_n = min((k + 1) * SH, N)
        iv = np.ones(SH, np.float32)
        iv[: hi_n - lo_n] = invdeg[lo_n:hi_n]
        S = np.zeros((P, TILES * NB, P), BF16)
        pidx, jidx = np.nonzero(dselc >= 0)
        cidx = dselc[pidx, jidx]
        tl = jidx // NB
        S[pidx, jidx, cidx] = iv[tl * P + cidx].astype(BF16)
        d = dict(
            idx_lo=_wrap_idx(idx_lo.astype(np.int16)),
            idx_hi=_wrap_idx(idx_hi.astype(np.int16)),
            S=np.ascontiguousarray(S.reshape(P, TILES * NB * P)),
        )
        d["shard_len"] = hi_n - lo_n
        per_core.append(d)
    return B2L, B2H, per_core


# ------------------------------- bass program -------------------------------

def _build(B2L, B2H):
    import concourse.bass as bass
    import concourse.bacc as bacc
    import concourse.tile as tile
    import concourse.mybir as mybir

    dt = mybir.dt
    Alu = mybir.AluOpType
    Act = mybir.ActivationFunctionType

    nc = bacc.Bacc("TRN2", target_bir_lowering=False, debug=False,
                   num_devices=NCORES, num_swdge_queues=NQ,
                   dynamic_dma_scratch_size=SCRATCH)

    NB = B2L + B2H

    # ---------------- I/O ----------------
    def inp(name, shape, d):
        return nc.dram_tensor(name, list(shape), d, kind="ExternalInput")

    table0 = inp("table0", [NTAB, P], dt.bfloat16)
    xTown = inp("xTown", [FIN, SH], dt.bfloat16)
    idx_lo = inp("idx_lo", [P, TILES * B2L * 8], dt.int16)
    idx_hi = inp("idx_hi", [P, TILES * B2H * 8], dt.int16)
    S_dram = inp("S", [P, TILES * NB * P], dt.bfloat16)
    identb = inp("identb", [H, H], dt.bfloat16)
    ones1 = inp("ones1", [1, P], dt.bfloat16)
    Wl0b = inp("Wl0b", [FIN, H], dt.bfloat16)
    Wl1b = inp("Wl1b", [H, H], dt.bfloat16)
    Wl2b = inp("Wl2b", [H, OUT], dt.bfloat16)
    Wr0b = inp("Wr0b", [FIN, H], dt.bfloat16)
    Wr1b = inp("Wr1b", [H, H], dt.bfloat16)
    Wr2b = inp("Wr2b", [H, OUT], dt.bfloat16)
    bl0c = inp("bl0c", [H, 1], dt.float32)
    bl1c = inp("bl1c", [H, 1], dt.float32)
    bl2c = inp("bl2c", [OUT, 1], dt.float32)
    g0c = inp("g0c", [H, 1], dt.float32)
    b0c = inp("b0c", [H, 1], dt.float32)
    g1c = inp("g1c", [H, 1], dt.float32)
    b1c = inp("b1c", [H, 1], dt.float32)

    out = nc.dram_tensor("out", [OUT, SH], dt.float32, kind="ExternalOutput")

    # DRAM scratch
    shard0 = nc.dram_tensor("shard0", [SH, P], dt.bfloat16)
    shard1 = nc.dram_tensor("shard1", [SH, P], dt.bfloat16)
    table1 = nc.dram_tensor("table1", [NTAB, P], dt.bfloat16, addr_space="Shared")
    table2 = nc.dram_tensor("table2", [NTAB, P], dt.bfloat16, addr_space="Shared")
    stats_in0 = nc.dram_tensor("stats_in0", [H, 2], dt.float32)
    stats_in1 = nc.dram_tensor("stats_in1", [H, 2], dt.float32)
    stats_out0 = nc.dram_tensor("stats_out0", [H, 2], dt.float32, addr_space="Shared")
    stats_out1 = nc.dram_tensor("stats_out1", [H, 2], dt.float32, addr_space="Shared")
    warm_in = nc.dram_tensor("warm_in", [H, 2], dt.float32)
    warm_out = nc.dram_tensor("warm_out", [H, 2], dt.float32, addr_space="Shared")

    layers = [
        dict(table=table0, Wl=Wl0b, Wr=Wr0b, bl=bl0c, KA=FIN,
             g=g0c, b=b0c, HO=H, sin=stats_in0, sout=stats_out0,
             shard=shard0, tnext=table1),
        dict(table=table1, Wl=Wl1b, Wr=Wr1b, bl=bl1c, KA=H,
             g=g1c, b=b1c, HO=H, sin=stats_in1, sout=stats_out1,
             shard=shard1, tnext=table2),
        dict(table=table2, Wl=Wl2b, Wr=Wr2b, bl=bl2c, KA=H,
             g=None, b=None, HO=OUT, sin=None, sout=None,
             shard=None, tnext=None),
    ]

    def allreduce(src, dst):
        nc.gpsimd.collective_compute(
            "AllReduce", Alu.add, replica_groups=[list(range(NCORES))],
            ins=[src.ap().opt()], outs=[dst.ap().opt()])

    with tile.TileContext(nc) as tc:
        with tc.tile_pool(name="const", bufs=1) as cpool, \
             tc.tile_pool(name="work", bufs=3) as wpool, \
             tc.tile_pool(name="outp", bufs=2) as opool, \
             tc.tile_pool(name="msgp", bufs=3) as mpool, \
             tc.tile_pool(name="msgt", bufs=1) as mtpool, \
             tc.tile_pool(name="spool", bufs=2) as spool, \
             tc.tile_pool(name="spoolt", bufs=1) as spoolt, \
             tc.tile_pool(name="psum", bufs=2, space="PSUM") as pp, \
             tc.tile_pool(name="psumb", bufs=1, space="PSUM") as ppb:

            def load_const(t):
                sl = tuple(slice(0, s) for s in t.shape)
                tl = cpool.tile(list(t.shape), t.dtype, tag=t.name,
                                name=f"c_{t.name}")
                nc.sync.dma_start(out=tl[:], in_=t[sl])
                return tl

            xTown_t = load_const(xTown)
            idxlo_t = load_const(idx_lo)
            idxhi_t = load_const(idx_hi)
            identb_t = load_const(identb)
            ones1_t = load_const(ones1)
            W_t = {l: tuple(load_const(t) for t in ts)
                   for l, ts in {0: (Wl0b, Wr0b, bl0c),
                                 1: (Wl1b, Wr1b, bl1c),
                                 2: (Wl2b, Wr2b, bl2c)}.items()}
            bn_t = {0: (load_const(g0c), load_const(b0c)),
                    1: (load_const(g1c), load_const(b1c))}

            # collective warmup (result unused; overlaps layer-0 gathers)
            if not SKIP_COLLECTIVES:
                allreduce(warm_in, warm_out)

            # persistent regions
            yT_l = {0: cpool.tile([H, SH], dt.bfloat16, tag="yTA",
                                  name="yTA"),
                    1: cpool.tile([H, SH], dt.bfloat16, tag="yTB",
                                  name="yTB")}
            NCH = len(CHUNKS)
            ssum = cpool.tile([H, NCH], dt.float32, tag="ssum")
            ssq = cpool.tile([H, NCH], dt.float32, tag="ssq")
            # deferred-BN state produced at each boundary (l = 0, 1)
            bn_state = {}

            for l, L in enumerate(layers):
                table = L["table"]
                Wl_t, Wr_t, bl_t = W_t[l]
                KA = L["KA"]
                HO = L["HO"]
                qctr = 0
                if l > 0:
                    st = bn_state[l - 1]
                for (t0, t1) in CHUNKS:
                    ct = t1 - t0
                    pool = mtpool if ct == 1 else mpool
                    msgL = pool.tile([P, ct * B2L, P], dt.bfloat16,
                                     tag=f"msgL{ct}")
                    msgH = pool.tile([P, ct * B2H, P], dt.bfloat16,
                                     tag=f"msgH{ct}")
                    # gathers split in half per lo/hi, spread over 4 queues
                    for (msg, idxs_t, bpt) in ((msgL, idxlo_t, B2L),
                                               (msgH, idxhi_t, B2H)):
                        nblk_tot = ct * bpt
                        hsp = [(0, nblk_tot)] if ct == 1 else \
                            [(0, nblk_tot // 2), (nblk_tot // 2, nblk_tot)]
                        for (b0_, b1_) in hsp:
                            nb = b1_ - b0_
                            ni = nb * P
                            i0 = (t0 * bpt + b0_) * 8
                            nc.gpsimd.dma_gather(
                                out_ap=msg[:, b0_:b1_, :],
                                in_ap=(table[0:SPLIT, :] if msg is msgL
                                       else table[SPLIT:NTAB, :]),
                                idxs_ap=idxs_t[:, i0:i0 + nb * 8],
                                num_idxs=ni, num_idxs_reg=ni, elem_size=P,
                                single_packet=False, queue_num=qctr % NQ)
                            qctr += 1
                            if l > 0:
                                # deferred BN+relu per gathered half:
                                #   msg = max(msg + theta, 0) on cols 0:KA
                                st_ = bn_state[l - 1]
                                th_b = st_["th_rep"][:, 0:KA].unsqueeze(1) \
                                    .broadcast_to([P, nb, KA])
                                nc.vector.tensor_tensor(
                                    out=msg[:, b0_:b1_, 0:KA],
                                    in0=msg[:, b0_:b1_, 0:KA],
                                    in1=th_b, op=Alu.add)
                                nc.vector.tensor_scalar(
                                    out=msg[:, b0_:b1_, 0:KA],
                                    in0=msg[:, b0_:b1_, 0:KA],
                                    scalar1=0.0, scalar2=None, op0=Alu.max)
                    # one-hot S (invdeg folded), streamed from HBM
                    St = (spool if ct > 1 else spoolt).tile(
                        [P, ct * NB, P], dt.bfloat16, tag=f"S{ct}",
                        name=f"S{ct}")
                    nc.sync.dma_start(
                        out=St[:],
                        in_=S_dram[:, t0 * NB * P:t1 * NB * P])
                    if l > 0:
                        # own-side: relu(y_prev + theta) feature-major
                        own_c = wpool.tile([H, ct * P], dt.bfloat16,
                                           tag=f"own{ct}")
                        nc.scalar.activation(
                            out=own_c[:], in_=yT_l[l - 1][:, t0 * P:t1 * P],
                            func=Act.Relu, bias=st["th_col"][:, 0:1])
                        Wl_use, Wr_use = st["Wlp"], st["Wrp"]
                    else:
                        Wl_use, Wr_use = Wl_t, Wr_t
                    ci = CHUNKS.index((t0, t1))
                    # batched PSUM: all ct tiles' agg chains in one bank
                    aggc = pp.tile([P, 4 * P], dt.float32, tag="aggc")
                    for t in range(t0, t1):
                        tt = t - t0          # chunk-local tile index
                        for b in range(NB):
                            if b < B2L:
                                mblk = msgL[:, tt * B2L + b, 0:KA]
                            else:
                                mblk = msgH[:, tt * B2H + (b - B2L), 0:KA]
                            nc.tensor.matmul(
                                out=aggc[0:KA, tt * P:(tt + 1) * P],
                                lhsT=mblk, rhs=St[:, tt * NB + b, :],
                                start=(b == 0), stop=(b == NB - 1))
                    # cast (mean already folded into S), one op per chunk
                    aggb = wpool.tile([KA, ct * P], dt.bfloat16,
                                      tag=f"aggb{KA}_{ct}",
                                      name=f"aggb{KA}_{ct}")
                    nc.vector.tensor_copy(out=aggb[:],
                                          in_=aggc[0:KA, 0:ct * P])
                    # dense: yT = Wl^T aggb + Wr^T own (+ bias in ACT)
                    ypsc = pp.tile([OUT, 4 * P], dt.float32, tag="ypsc")
                    for t in range(t0, t1):
                        tt = t - t0
                        nc.tensor.matmul(out=ypsc[0:HO, tt * P:(tt + 1) * P],
                                         lhsT=Wl_use[:],
                                         rhs=aggb[:, tt * P:(tt + 1) * P],
                                         start=True, stop=False)
                        rhs_own = (own_c[:, tt * P:(tt + 1) * P] if l > 0
                                   else xTown_t[:, t * P:(t + 1) * P])
                        nc.tensor.matmul(out=ypsc[0:HO, tt * P:(tt + 1) * P],
                                         lhsT=Wr_use[:], rhs=rhs_own,
                                         start=False, stop=True)
                    if l < 2:
                        yT = yT_l[l]
                        nc.scalar.activation(
                            out=yT[:, t0 * P:t1 * P],
                            in_=ypsc[0:H, 0:ct * P], func=Act.Identity,
                            bias=bl_t[:, 0:1],
                            accum_out=ssum[:, ci:ci + 1])
                        sq = wpool.tile([H, ct * P], dt.float32,
                                        tag=f"sq{ct}")
                        nc.scalar.activation(
                            out=sq[:], in_=ypsc[0:H, 0:ct * P], func=Act.Square,
                            bias=bl_t[:, 0:1],
                            accum_out=ssq[:, ci:ci + 1])
                        # transpose + shard write (pre-BN y), per chunk
                        y_chunk = opool.tile([P, ct, P], dt.bfloat16,
                                             tag=f"yc{ct}")
                        nc.vector.memset(y_chunk[:, :, H:P], 0.0)
                        for t in range(t0, t1):
                            ptr = pp.tile([P, H], dt.bfloat16, tag="ps_tr")
                            nc.tensor.transpose(
                                out=ptr[:], in_=yT[:, t * P:(t + 1) * P],
                                identity=identb_t[:])
                            nc.scalar.activation(
                                out=y_chunk[:, t - t0, 0:H], in_=ptr[:],
                                func=Act.Copy)
                        nc.scalar.dma_start(
                            out=L["shard"][t0 * P:t1 * P, :].rearrange(
                                "(t p) d -> p t d", p=P),
                            in_=y_chunk[:])
                    else:
                        y2c = opool.tile([OUT, ct * P], dt.float32,
                                         tag=f"y2c{ct}")
                        nc.scalar.activation(
                            out=y2c[:], in_=ypsc[0:OUT, 0:ct * P],
                            func=Act.Identity, bias=bl_t[:, 0:1])
                        nc.scalar.dma_start(out=out[:, t0 * P:t1 * P],
                                            in_=y2c[:])

                if l < 2:
                    # ---- table AllGather first (no stats dependency), then
                    # ---- BN stats AllReduce right behind it on the CC stream
                    if SKIP_COLLECTIVES:
                        nc.sync.dma_start(out=L["tnext"][0:SH, 0:P],
                                          in_=L["shard"][0:SH, 0:P])
                    else:
                        nc.gpsimd.collective_compute(
                            "AllGather", Alu.bypass,
                            replica_groups=[list(range(NCORES))],
                            ins=[L["shard"].ap().opt()],
                            outs=[L["tnext"].ap().opt()])
                    stats = wpool.tile([H, 2], dt.float32, tag="stats")
                    nc.vector.tensor_reduce(out=stats[:, 0:1], in_=ssum[:],
                                            axis=mybir.AxisListType.X, op=Alu.add)
                    nc.vector.tensor_reduce(out=stats[:, 1:2], in_=ssq[:],
                                            axis=mybir.AxisListType.X, op=Alu.add)
                    nc.sync.dma_start(out=L["sin"][0:H, 0:2], in_=stats[:])
                    if SKIP_COLLECTIVES:
                        nc.sync.dma_start(out=L["sout"][0:H, 0:2],
                                          in_=L["sin"][0:H, 0:2])
                    else:
                        allreduce(L["sin"], L["sout"])
                    sg = wpool.tile([H, 2], dt.float32, tag="sg")
                    nc.sync.dma_start(out=sg[:], in_=L["sout"][0:H, 0:2])
                    # s = g/sqrt(var+eps); t = b - mu*s; theta = t/s
                    mu = wpool.tile([H, 1], dt.float32, tag="mu")
                    nc.vector.tensor_scalar(out=mu[:], in0=sg[:, 0:1],
                                            scalar1=1.0 / N, scalar2=None,
                                            op0=Alu.mult)
                    var = wpool.tile([H, 1], dt.float32, tag="var")
                    nc.vector.tensor_scalar(out=var[:], in0=sg[:, 1:2],
                                            scalar1=1.0 / N, scalar2=None,
                                            op0=Alu.mult)
                    mu2 = wpool.tile([H, 1], dt.float32, tag="mu2")
                    nc.vector.tensor_tensor(out=mu2[:], in0=mu[:], in1=mu[:],
                                            op=Alu.mult)
                    nc.vector.tensor_tensor(out=var[:], in0=var[:], in1=mu2[:],
                                            op=Alu.subtract)
                    nc.vector.tensor_scalar(out=var[:], in0=var[:],
                                            scalar1=float(EPS), scalar2=None,
                                            op0=Alu.add)
                    std = wpool.tile([H, 1], dt.float32, tag="std")
                    nc.scalar.activation(out=std[:], in_=var[:], func=Act.Sqrt)
                    istd = wpool.tile([H, 1], dt.float32, tag="istd")
                    nc.vector.reciprocal(out=istd[:], in_=std[:])
                    g_t, bb_t = bn_t[l]
                    s_col = cpool.tile([H, 1], dt.float32, tag=f"s_col{l}")
                    nc.vector.tensor_tensor(out=s_col[:], in0=g_t[:],
                                            in1=istd[:], op=Alu.mult)
                    ms = wpool.tile([H, 1], dt.float32, tag="ms")
                    nc.vector.tensor_tensor(out=ms[:], in0=mu[:], in1=s_col[:],
                                            op=Alu.mult)
                    t_col = wpool.tile([H, 1], dt.float32, tag="t_col")
                    nc.vector.tensor_tensor(out=t_col[:], in0=bb_t[:],
                                            in1=ms[:], op=Alu.subtract)
                    sinv = wpool.tile([H, 1], dt.float32, tag="sinv")
                    nc.vector.reciprocal(out=sinv[:], in_=s_col[:])
                    th_col = cpool.tile([H, 1], dt.float32, tag=f"th_col{l}")
                    nc.vector.tensor_tensor(out=th_col[:], in0=t_col[:],
                                            in1=sinv[:], op=Alu.mult)
                    # theta replicated to all partitions: ones1^T (theta^T)
                    th_cb = wpool.tile([H, 1], dt.bfloat16, tag="th_cb")
                    nc.vector.tensor_copy(out=th_cb[:], in_=th_col[:])
                    trps = ppb.tile([1, H], dt.float32, tag="trps")
                    nc.tensor.matmul(out=trps[:], lhsT=th_cb[:],
                                     rhs=identb_t[:], start=True, stop=True)
                    th_row = wpool.tile([1, H], dt.bfloat16, tag="th_row")
                    nc.vector.tensor_copy(out=th_row[:], in_=trps[:])
                    threp_ps = ppb.tile([P, H], dt.float32, tag="threp_ps")
                    nc.tensor.matmul(out=threp_ps[:], lhsT=ones1_t[:],
                                     rhs=th_row[:], start=True, stop=True)
                    th_rep = cpool.tile([P, H], dt.bfloat16, tag=f"th_rep{l}")
                    nc.vector.tensor_copy(out=th_rep[:], in_=threp_ps[:])
                    # fold s into the next layer's weights (rows = features)
                    Wln, Wrn, _ = W_t[l + 1]
                    HO_n = layers[l + 1]["HO"]
                    Wlp = cpool.tile([H, HO_n], dt.bfloat16, tag=f"Wlp{l}")
                    nc.vector.tensor_scalar(out=Wlp[:], in0=Wln[:],
                                            scalar1=s_col[:, 0:1],
                                            scalar2=None, op0=Alu.mult)
                    Wrp = cpool.tile([H, HO_n], dt.bfloat16, tag=f"Wrp{l}")
                    nc.vector.tensor_scalar(out=Wrp[:], in0=Wrn[:],
                                            scalar1=s_col[:, 0:1],
                                            scalar2=None, op0=Alu.mult)
                    bn_state[l] = dict(th_col=th_col, th_rep=th_rep,
                                       Wlp=Wlp, Wrp=Wrp)

    nc.compile()
    return nc


# --------------------------------- runner -----------------------------------

def _get_nc(B2L, B2H):
    key = (B2L, B2H)
    if key not in _CACHE:
        _CACHE[key] = _build(B2L, B2H)
    return _CACHE[key]


def make_in_maps(x, Wl0, bl0, Wr0, g0, b0, Wl1, bl1, Wr1, g1, b1,
                 Wl2, bl2, Wr2, per_core):
    x = np.asarray(x, np.float32)
    tab0 = np.zeros((NTAB, P), np.float32)
    tab0[:N, :FIN] = x
    tab0 = tab0.astype(BF16)
    xTfull = np.zeros((FIN, NTAB), np.float32)
    xTfull[:, :N] = x.T
    xTb = xTfull.astype(BF16)

    common = dict(
        table0=tab0,
        identb=np.eye(H, dtype=np.float32).astype(BF16),
        ones1=np.ones((1, P), np.float32).astype(BF16),
        Wl0b=np.asarray(Wl0, np.float32).astype(BF16),
        Wl1b=np.asarray(Wl1, np.float32).astype(BF16),
        Wl2b=np.asarray(Wl2, np.float32).astype(BF16),
        Wr0b=np.asarray(Wr0, np.float32).astype(BF16),
        Wr1b=np.asarray(Wr1, np.float32).astype(BF16),
        Wr2b=np.asarray(Wr2, np.float32).astype(BF16),
        bl0c=np.ascontiguousarray(np.asarray(bl0, np.float32).reshape(H, 1)),
        bl1c=np.ascontiguousarray(np.asarray(bl1, np.float32).reshape(H, 1)),
        bl2c=np.ascontiguousarray(np.asarray(bl2, np.float32).reshape(OUT, 1)),
        g0c=np.ascontiguousarray(np.asarray(g0, np.float32).reshape(H, 1)),
        b0c=np.ascontiguousarray(np.asarray(b0, np.float32).reshape(H, 1)),
        g1c=np.ascontiguousarray(np.asarray(g1, np.float32).reshape(H, 1)),
        b1c=np.ascontiguousarray(np.asarray(b1, np.float32).reshape(H, 1)),
        warm_in=np.zeros((H, 2), np.float32),
    )

    in_maps = []
    for k in range(NCORES):
        d = per_core[k]
        m = dict(common)
        m["xTown"] = np.ascontiguousarray(xTb[:, k * SH:(k + 1) * SH])
        for key in ("idx_lo", "idx_hi", "S"):
            m[key] = d[key]
        in_maps.append(m)
    return in_maps


def run(inputs, trace=False):
    """Build+run; returns (full_output, BassKernelResults)."""
    _install_ntff_shim()
    from concourse import bass_utils

    edge_index = np.asarray(inputs["edge_index"])
    B2L, B2H, per_core = _prep(edge_index)
    nc = _get_nc(B2L, B2H)
    in_maps = make_in_maps(
        inputs["x"], inputs["Wl0"], inputs["bl0"], inputs["Wr0"],
        inputs["g0"], inputs["b0"], inputs["Wl1"], inputs["bl1"],
        inputs["Wr1"], inputs["g1"], inputs["b1"], inputs["Wl2"],
        inputs["bl2"], inputs["Wr2"], per_core)
    res = bass_utils.run_bass_kernel_spmd(nc, in_maps,
                                          core_ids=list(range(NCORES)),
                                          trace=trace)
    parts = []
    for k in range(NCORES):
        n_k = per_core[k]["shard_len"]
        parts.append(res.results[k]["out"][:, :n_k].T)
    full = np.ascontiguousarray(np.concatenate(parts, axis=0),
                                dtype=np.float32)
    return full, res


def kernel(x, edge_index, Wl0, bl0, Wr0, g0, b0, Wl1, bl1, Wr1, g1, b1,
           Wl2, bl2, Wr2):
    full, _ = run(dict(x=x, edge_index=edge_index, Wl0=Wl0, bl0=bl0, Wr0=Wr0,
                       g0=g0, b0=b0, Wl1=Wl1, bl1=bl1, Wr1=Wr1, g1=g1, b1=b1,
                       Wl2=Wl2, bl2=bl2, Wr2=Wr2))
    return full
